# revision 8
# baseline (speedup 1.0000x reference)
"""Trainium2 Bass kernel for single-head causal attention.

Problem: B=4, T=4096, C=768, fp32.
  Q = x@Wq+bq; K = x@Wk+bk; V = x@Wv+bv
  out = softmax(causal(Q K^T / sqrt(C))) @ V

Sharding (8 cores): 2 cores per batch element. Each core processes ALL 4096
queries of its batch but only HALF the key tiles (128-row tiles, interleaved
by parity m = core%2). This makes the instruction stream identical across
cores (required for SPMD: one NEFF, data-only differences) and splits the
causal flash-attention work exactly 50/50 at i-block granularity of 256.

Each core returns unnormalized O_m = sum_j exp(s_ij) v_j and l_m = sum_j
exp(s_ij) (ones-column trick appended to V). Host combines:
  out = (O_0 + O_1) / (l_0 + l_1) + bv
(bv folds out of the attention average since softmax rows sum to 1;
no max-subtraction needed: |scores| <= ~5 so exp is well-conditioned.)

Matmuls run in float32r (TF32-class, ~1e-4 rel err, full PE rate at
free-dim >= 256). Producers must round to f32r explicitly.
"""
import sys

sys.path.insert(0, "/opt/trn_rl_repo")

import numpy as np
from contextlib import ExitStack

import concourse.bass as bass
import concourse.bacc as bacc
import concourse.mybir as mybir
import concourse.tile as tile
from concourse.bass_utils import run_bass_kernel_spmd
from concourse.masks import make_identity

dt = mybir.dt
F32, F32R = dt.float32, dt.float32r
AFT = mybir.ActivationFunctionType

B, T, C = 4, 4096, 768
NCK = C // 128            # 6 contraction tiles
NKT = T // 2 // 128       # 16 key tiles per core
NQ4 = T // 4              # 1024 queries per quarter-pass
SCALE = 1.0 / float(np.sqrt(np.float32(C)))

_nc_cache = {}
last_exec_time_ns = None
last_results = None


def _transpose_block(nc, ps_tr, xt_dst, x_src, eng_sel, ident):
    """PE-transpose x_src [128,768] f32 -> xt_dst view [128, 6, 128] f32r.

    xt_dst is an AP [128, 6, 128] (plane-strided dest). Routes the two
    PSUM evictions to alternating engines via eng_sel (0/1).
    """
    pt = ps_tr.tile([128, 512], F32, tag="tr")
    for k in range(4):
        nc.tensor.matmul(pt[:, 128 * k:128 * k + 128],
                         lhsT=x_src[:, 128 * k:128 * k + 128], rhs=ident[:],
                         is_transpose=True, start=(k == 0), stop=(k == 3))
    if eng_sel == 0:
        nc.scalar.activation(xt_dst[:, 0:4, :], pt[:].rearrange("p (k f) -> p k f", k=4), AFT.Identity)
    else:
        nc.vector.tensor_copy(xt_dst[:, 0:4, :], pt[:].rearrange("p (k f) -> p k f", k=4))
    pt2 = ps_tr.tile([128, 512], F32, tag="tr")
    for k in range(2):
        nc.tensor.matmul(pt2[:, 128 * k:128 * k + 128],
                         lhsT=x_src[:, 512 + 128 * k:512 + 128 * k + 128], rhs=ident[:],
                         is_transpose=True, start=(k == 0), stop=(k == 1))
    if eng_sel == 0:
        nc.vector.tensor_copy(xt_dst[:, 4:6, :], pt2[:, 0:256].rearrange("p (k f) -> p k f", k=2))
    else:
        nc.scalar.activation(xt_dst[:, 4:6, :], pt2[:, 0:256].rearrange("p (k f) -> p k f", k=2), AFT.Identity)


def build_module():
    nc = bacc.Bacc("TRN2", target_bir_lowering=False, debug=False)

    xq = nc.dram_tensor("xq", [T, C], F32, kind="ExternalInput").ap()
    xk = nc.dram_tensor("xk", [T // 2, C], F32, kind="ExternalInput").ap()
    wq = nc.dram_tensor("wq", [C, C], F32, kind="ExternalInput").ap()
    wk = nc.dram_tensor("wk", [C, C], F32, kind="ExternalInput").ap()
    wv = nc.dram_tensor("wv", [C, C], F32, kind="ExternalInput").ap()
    bq = nc.dram_tensor("bq", [C], F32, kind="ExternalInput").ap()
    bk = nc.dram_tensor("bk", [C], F32, kind="ExternalInput").ap()
    msk = nc.dram_tensor("msk", [128, 256], F32, kind="ExternalInput").ap()
    out = nc.dram_tensor("out", [T, C + 1], F32, kind="ExternalOutput").ap()

    with tile.TileContext(nc) as tc, ExitStack() as ctx:
        const = ctx.enter_context(tc.tile_pool(name="const", bufs=1))
        ident = const.tile([128, 128], F32)
        make_identity(nc, ident[:])
        mask_sb = const.tile([128, 256], F32R)
        bq_sb = const.tile([128, NCK], F32)
        nc.sync.dma_start(bq_sb[:], bq.rearrange("(k p) -> p k", p=128))
        bk_sb = const.tile([128, NCK], F32)
        nc.sync.dma_start(bk_sb[:], bk.rearrange("(k p) -> p k", p=128))
        onez = const.tile([128, 2], F32)
        nc.vector.memset(onez[:, 0:1], 1.0)
        nc.vector.memset(onez[:, 1:2], 0.0)

        # --- weights: load fp32, round to f32r, plane layout [128, ck, 768]
        wq_pool = ctx.enter_context(tc.tile_pool(name="wq", bufs=1))
        wq_r = wq_pool.tile([128, NCK * C], F32R)
        wkv_pool = ctx.enter_context(tc.tile_pool(name="wkv", bufs=1))
        wk_r = wkv_pool.tile([128, NCK * C], F32R)
        wv_r = wkv_pool.tile([128, NCK * C], F32R)
        with tc.tile_pool(name="wstage", bufs=1) as wstage:
            mf = wstage.tile([128, 256], F32)
            nc.sync.dma_start(mf[:], msk[:])
            nc.vector.tensor_copy(mask_sb[:], mf[:])
            for w_dram, w_dst in ((wq, wq_r), (wk, wk_r), (wv, wv_r)):
                w_f32 = wstage.tile([128, NCK * C], F32, tag="wst")
                nc.sync.dma_start(
                    w_f32[:].rearrange("p (k n) -> p k n", k=NCK),
                    w_dram.rearrange("(k p) n -> p k n", p=128))
                nc.vector.tensor_copy(w_dst[:], w_f32[:])

        # --- PSUM pools (8 banks total):
        # tr: 1 bank, proj: 2, st: 1, O: 4
        ps_tr = ctx.enter_context(tc.tile_pool(name="ps_tr", bufs=1, space="PSUM"))
        ps_pj = ctx.enter_context(tc.tile_pool(name="ps_pj", bufs=2, space="PSUM"))
        ps_st = ctx.enter_context(tc.tile_pool(name="ps_st", bufs=1, space="PSUM"))
        ps_o = ctx.enter_context(tc.tile_pool(name="ps_o", bufs=1, space="PSUM"))

        kt_pool = ctx.enter_context(tc.tile_pool(name="kt", bufs=1))
        v_pool = ctx.enter_context(tc.tile_pool(name="v", bufs=1))
        # KT planes: [128, dk, 2048]; V tiles: [128, t, 769] (col 768 = ones)
        kt_sb = kt_pool.tile([128, NCK * 2048], F32R)
        v_sb = v_pool.tile([128, NKT * 770], F32R)

        # ---------------- phase K: keys -> KT, V ----------------
        with tc.tile_pool(name="xkst", bufs=2) as xkst, \
             tc.tile_pool(name="xkt", bufs=2) as xktp:
            for win in range(4):        # 512-key windows
                xkt = xktp.tile([128, NCK * 512], F32R, tag="xktw")
                for tt in range(4):     # 128-key tiles in window
                    x_sb = xkst.tile([128, C], F32, tag="xk")
                    nc.sync.dma_start(x_sb[:], xk[512 * win + 128 * tt: 512 * win + 128 * tt + 128, :])
                    xt_view = xkt[:].rearrange("p (k n) -> p k n", k=NCK)[:, :, 128 * tt:128 * tt + 128]
                    _transpose_block(nc, ps_tr, xt_view, x_sb, tt % 2, ident)
                # KT projection for this window: KT[dk, 512*win:+512]
                for co in range(NCK):
                    pj = ps_pj.tile([128, 512], F32, tag="pj")
                    for ck in range(NCK):
                        nc.tensor.matmul(
                            pj[:],
                            lhsT=wk_r[:, C * ck + 128 * co: C * ck + 128 * co + 128],
                            rhs=xkt[:, 512 * ck: 512 * ck + 512],
                            start=(ck == 0), stop=(ck == NCK - 1))
                    nc.scalar.activation(kt_sb[:, 2048 * co + 512 * win: 2048 * co + 512 * win + 512],
                                         pj[:], AFT.Identity, bias=bk_sb[:, co:co + 1])
                # V projection for the 4 tiles in this window (no bias)
                for tt in range(4):
                    t_glob = 4 * win + tt
                    for half in range(2):
                        n0, nn = (0, 512) if half == 0 else (512, 256)
                        pj = ps_pj.tile([128, 512], F32, tag="pj")
                        for ck in range(NCK):
                            nc.tensor.matmul(
                                pj[:, 0:nn],
                                lhsT=xkt[:, 512 * ck + 128 * tt: 512 * ck + 128 * tt + 128],
                                rhs=wv_r[:, C * ck + n0: C * ck + n0 + nn],
                                start=(ck == 0), stop=(ck == NCK - 1))
                        if half == 0:
                            nc.vector.tensor_copy(v_sb[:, 770 * t_glob: 770 * t_glob + 512], pj[:, 0:512])
                        else:
                            nc.scalar.activation(v_sb[:, 770 * t_glob + 512: 770 * t_glob + 768],
                                                 pj[:, 0:256], AFT.Identity)
                    nc.vector.tensor_copy(v_sb[:, 770 * t_glob + 768: 770 * t_glob + 770], onez[:])

        # ---------------- phase Q: quarters of queries ----------------
        with tc.tile_pool(name="xqst", bufs=2) as xqst, \
             tc.tile_pool(name="xqt", bufs=2) as xqtp, \
             tc.tile_pool(name="qt", bufs=1) as qtp, \
             tc.tile_pool(name="pt", bufs=2) as ptp, \
             tc.tile_pool(name="ob", bufs=1) as obp:
            for widx in range(8):       # 512-query windows
                qt_sb = qtp.tile([128, NCK * 512], F32R, tag="qt")
                xqt = xqtp.tile([128, NCK * 512], F32R, tag="xqtw")
                for tt in range(4):
                    r0 = 512 * widx + 128 * tt
                    x_sb = xqst.tile([128, C], F32, tag="xq")
                    nc.sync.dma_start(x_sb[:], xq[r0:r0 + 128, :])
                    xt_view = xqt[:].rearrange("p (k n) -> p k n", k=NCK)[:, :, 128 * tt:128 * tt + 128]
                    _transpose_block(nc, ps_tr, xt_view, x_sb, tt % 2, ident)
                for co in range(NCK):
                    pj = ps_pj.tile([128, 512], F32, tag="pj")
                    for ck in range(NCK):
                        nc.tensor.matmul(
                            pj[:],
                            lhsT=wq_r[:, C * ck + 128 * co: C * ck + 128 * co + 128],
                            rhs=xqt[:, 512 * ck: 512 * ck + 512],
                            start=(ck == 0), stop=(ck == NCK - 1))
                    nc.scalar.activation(qt_sb[:, 512 * co: 512 * co + 512],
                                         pj[:], AFT.Identity, bias=bq_sb[:, co:co + 1])
                # flash over i-blocks of 256
                for al in range(2):
                    a = 2 * widx + al        # global i-block; queries 256a..256a+255
                    o_ps = []
                    for s2 in range(2):
                        oa_t = ps_o.tile([128, 512], F32, tag=f"oa{s2}", name=f"oa{s2}_{a}")
                        ob_t = ps_o.tile([128, 258], F32, tag=f"ob{s2}", name=f"ob{s2}_{a}")
                        o_ps.append((oa_t, ob_t))
                    for t in range(a + 1):   # local key tiles (global tile 2t+m)
                        st = ps_st.tile([128, 256], F32, tag="st")
                        for dk in range(NCK):
                            nc.tensor.matmul(
                                st[:],
                                lhsT=kt_sb[:, 2048 * dk + 128 * t: 2048 * dk + 128 * t + 128],
                                rhs=qt_sb[:, 512 * dk + 256 * al: 512 * dk + 256 * al + 256],
                                start=(dk == 0), stop=(dk == NCK - 1))
                        pt = ptp.tile([128, 256], F32R, tag="pt")
                        nc.scalar.activation(pt[:], st[:], AFT.Exp, scale=SCALE)
                        if t == a:
                            nc.vector.tensor_mul(pt[:], pt[:], mask_sb[:])
                        for s2 in range(2):
                            oa, ob = o_ps[s2]
                            nc.tensor.matmul(oa[:], lhsT=pt[:, 128 * s2:128 * s2 + 128],
                                             rhs=v_sb[:, 770 * t:770 * t + 512],
                                             start=(t == 0), stop=(t == a))
                            nc.tensor.matmul(ob[:], lhsT=pt[:, 128 * s2:128 * s2 + 128],
                                             rhs=v_sb[:, 770 * t + 512:770 * t + 770],
                                             start=(t == 0), stop=(t == a))
                    for s2 in range(2):
                        oa, ob = o_ps[s2]
                        o_sb = obp.tile([128, 770], F32, tag="osb")
                        nc.vector.tensor_copy(o_sb[:, 0:512], oa[:])
                        nc.scalar.activation(o_sb[:, 512:770], ob[:], AFT.Identity)
                        nc.sync.dma_start(out[256 * a + 128 * s2: 256 * a + 128 * s2 + 128, :], o_sb[:, 0:769])

    nc.compile()
    return nc


def _build_mask(m):
    jl = np.arange(128)[:, None]
    il = np.arange(256)[None, :]
    return (il >= jl + 128 * m).astype(np.float32)


def kernel(input, Wq, bq, Wk, bk, Wv, bv):
    global last_exec_time_ns, last_results
    x = np.ascontiguousarray(np.asarray(input, dtype=np.float32))
    Wq = np.ascontiguousarray(np.asarray(Wq, dtype=np.float32))
    Wk = np.ascontiguousarray(np.asarray(Wk, dtype=np.float32))
    Wv = np.ascontiguousarray(np.asarray(Wv, dtype=np.float32))
    bq = np.ascontiguousarray(np.asarray(bq, dtype=np.float32))
    bk = np.ascontiguousarray(np.asarray(bk, dtype=np.float32))
    bv_np = np.ascontiguousarray(np.asarray(bv, dtype=np.float32))

    if "nc" not in _nc_cache:
        _nc_cache["nc"] = build_module()
    nc = _nc_cache["nc"]

    masks = [_build_mask(m) for m in range(2)]
    key_rows = [np.concatenate([np.arange(128 * (2 * t + m), 128 * (2 * t + m) + 128)
                                for t in range(NKT)]) for m in range(2)]
    in_maps = []
    for core in range(8):
        b, m = core // 2, core % 2
        in_maps.append({
            "xq": x[b],
            "xk": np.ascontiguousarray(x[b][key_rows[m]]),
            "wq": Wq, "wk": Wk, "wv": Wv, "bq": bq, "bk": bk,
            "msk": masks[m],
        })

    trace = bool(int(__import__("os").environ.get("KERNEL_TRACE", "0")))
    res = run_bass_kernel_spmd(nc, in_maps, core_ids=list(range(8)), trace=trace)
    last_exec_time_ns = res.exec_time_ns
    last_results = res

    y = np.empty((B, T, C), dtype=np.float32)
    for b in range(B):
        o0 = res.results[2 * b]["out"]
        o1 = res.results[2 * b + 1]["out"]
        O = o0[:, :C].astype(np.float64) + o1[:, :C].astype(np.float64)
        l = o0[:, C].astype(np.float64) + o1[:, C].astype(np.float64)
        y[b] = (O / l[:, None] + bv_np.astype(np.float64)).astype(np.float32)
    return y


# revision 9
# speedup vs baseline: 1.0148x; 1.0148x over previous
"""Trainium2 Bass kernel for single-head causal attention.

Problem: B=4, T=4096, C=768, fp32.
  Q = x@Wq+bq; K = x@Wk+bk; V = x@Wv+bv
  out = softmax(causal(Q K^T / sqrt(C))) @ V

Sharding (8 cores): 2 cores per batch element. Each core processes ALL 4096
queries of its batch but only HALF the key tiles (128-row tiles, interleaved
by parity m = core%2). This makes the instruction stream identical across
cores (required for SPMD: one NEFF, data-only differences) and splits the
causal flash-attention work exactly 50/50 at i-block granularity of 256.

Each core returns unnormalized O_m = sum_j exp(s_ij) v_j and l_m = sum_j
exp(s_ij) (ones-column trick appended to V). Host combines:
  out = (O_0 + O_1) / (l_0 + l_1) + bv
(bv folds out of the attention average since softmax rows sum to 1;
no max-subtraction needed: |scores| <= ~5 so exp is well-conditioned.)

Matmuls run in float32r (TF32-class, ~1e-4 rel err, full PE rate at
free-dim >= 256). Producers must round to f32r explicitly.
"""
import sys

sys.path.insert(0, "/opt/trn_rl_repo")

import numpy as np
from contextlib import ExitStack

import concourse.bass as bass
import concourse.bacc as bacc
import concourse.mybir as mybir
import concourse.tile as tile
from concourse.bass_utils import run_bass_kernel_spmd
from concourse.masks import make_identity

dt = mybir.dt
F32, F32R = dt.float32, dt.float32r
AFT = mybir.ActivationFunctionType

B, T, C = 4, 4096, 768
NCK = C // 128            # 6 contraction tiles
NKT = T // 2 // 128       # 16 key tiles per core
NQ4 = T // 4              # 1024 queries per quarter-pass
SCALE = 1.0 / float(np.sqrt(np.float32(C)))

_nc_cache = {}
last_exec_time_ns = None
last_results = None


def _transpose_block(nc, ps_tr, xt_dst, x_src, eng_sel, ident):
    """PE-transpose x_src [128,768] f32 -> xt_dst view [128, 6, 128] f32r.

    xt_dst is an AP [128, 6, 128] (plane-strided dest). Routes the two
    PSUM evictions to alternating engines via eng_sel (0/1).
    """
    pt = ps_tr.tile([128, 512], F32, tag="pj", name="trp")
    for k in range(4):
        nc.tensor.matmul(pt[:, 128 * k:128 * k + 128],
                         lhsT=x_src[:, 128 * k:128 * k + 128], rhs=ident[:],
                         is_transpose=True, start=(k == 0), stop=(k == 3))
    if eng_sel == 0:
        nc.scalar.activation(xt_dst[:, 0:4, :], pt[:].rearrange("p (k f) -> p k f", k=4), AFT.Identity)
    else:
        nc.vector.tensor_copy(xt_dst[:, 0:4, :], pt[:].rearrange("p (k f) -> p k f", k=4))
    pt2 = ps_tr.tile([128, 512], F32, tag="pj", name="trp2")
    for k in range(2):
        nc.tensor.matmul(pt2[:, 128 * k:128 * k + 128],
                         lhsT=x_src[:, 512 + 128 * k:512 + 128 * k + 128], rhs=ident[:],
                         is_transpose=True, start=(k == 0), stop=(k == 1))
    if eng_sel == 0:
        nc.vector.tensor_copy(xt_dst[:, 4:6, :], pt2[:, 0:256].rearrange("p (k f) -> p k f", k=2))
    else:
        nc.scalar.activation(xt_dst[:, 4:6, :], pt2[:, 0:256].rearrange("p (k f) -> p k f", k=2), AFT.Identity)


def build_module():
    nc = bacc.Bacc("TRN2", target_bir_lowering=False, debug=False)

    xq = nc.dram_tensor("xq", [T, C], F32, kind="ExternalInput").ap()
    xk = nc.dram_tensor("xk", [T // 2, C], F32, kind="ExternalInput").ap()
    wq = nc.dram_tensor("wq", [C, C], F32, kind="ExternalInput").ap()
    wk = nc.dram_tensor("wk", [C, C], F32, kind="ExternalInput").ap()
    wv = nc.dram_tensor("wv", [C, C], F32, kind="ExternalInput").ap()
    bq = nc.dram_tensor("bq", [C], F32, kind="ExternalInput").ap()
    bk = nc.dram_tensor("bk", [C], F32, kind="ExternalInput").ap()
    msk = nc.dram_tensor("msk", [128, 256], F32, kind="ExternalInput").ap()
    out = nc.dram_tensor("out", [T, C + 1], F32, kind="ExternalOutput").ap()

    with tile.TileContext(nc) as tc, ExitStack() as ctx:
        const = ctx.enter_context(tc.tile_pool(name="const", bufs=1))
        ident = const.tile([128, 128], F32)
        make_identity(nc, ident[:])
        mask_sb = const.tile([128, 256], F32R)
        bq_sb = const.tile([128, NCK], F32)
        nc.sync.dma_start(bq_sb[:], bq.rearrange("(k p) -> p k", p=128))
        bk_sb = const.tile([128, NCK], F32)
        nc.sync.dma_start(bk_sb[:], bk.rearrange("(k p) -> p k", p=128))
        onez = const.tile([128, 2], F32)
        nc.vector.memset(onez[:, 0:1], 1.0)
        nc.vector.memset(onez[:, 1:2], 0.0)

        # --- weights: load fp32, round to f32r, plane layout [128, ck, 768]
        wq_pool = ctx.enter_context(tc.tile_pool(name="wq", bufs=1))
        wq_r = wq_pool.tile([128, NCK * C], F32R)
        wkv_pool = ctx.enter_context(tc.tile_pool(name="wkv", bufs=1))
        wk_r = wkv_pool.tile([128, NCK * C], F32R)
        wv_r = wkv_pool.tile([128, NCK * C], F32R)
        with tc.tile_pool(name="wstage", bufs=1) as wstage:
            mf = wstage.tile([128, 256], F32)
            nc.sync.dma_start(mf[:], msk[:])
            nc.vector.tensor_copy(mask_sb[:], mf[:])
            for w_dram, w_dst in ((wq, wq_r), (wk, wk_r), (wv, wv_r)):
                w_f32 = wstage.tile([128, NCK * C], F32, tag="wst")
                nc.sync.dma_start(
                    w_f32[:].rearrange("p (k n) -> p k n", k=NCK),
                    w_dram.rearrange("(k p) n -> p k n", p=128))
                nc.vector.tensor_copy(w_dst[:], w_f32[:])

        # --- PSUM pools (8 banks total):
        # tr: 1 bank, proj: 2, st: 1, O: 4
        ps_pj = ctx.enter_context(tc.tile_pool(name="ps_pj", bufs=2, space="PSUM"))
        ps_st = ctx.enter_context(tc.tile_pool(name="ps_st", bufs=2, space="PSUM"))
        ps_o = ctx.enter_context(tc.tile_pool(name="ps_o", bufs=1, space="PSUM"))

        kt_pool = ctx.enter_context(tc.tile_pool(name="kt", bufs=1))
        v_pool = ctx.enter_context(tc.tile_pool(name="v", bufs=1))
        # KT planes: [128, dk, 2048]; V tiles: [128, t, 769] (col 768 = ones)
        kt_sb = kt_pool.tile([128, NCK * 2048], F32R)
        v_sb = v_pool.tile([128, NKT * 770], F32R)

        # ---------------- phase K: keys -> KT, V ----------------
        with tc.tile_pool(name="xkst", bufs=2) as xkst, \
             tc.tile_pool(name="xkt", bufs=2) as xktp:
            for win in range(4):        # 512-key windows
                xkt = xktp.tile([128, NCK * 512], F32R, tag="xktw")
                for tt in range(4):     # 128-key tiles in window
                    x_sb = xkst.tile([128, C], F32, tag="xk")
                    nc.scalar.dma_start(x_sb[:], xk[512 * win + 128 * tt: 512 * win + 128 * tt + 128, :])
                    xt_view = xkt[:].rearrange("p (k n) -> p k n", k=NCK)[:, :, 128 * tt:128 * tt + 128]
                    _transpose_block(nc, ps_pj, xt_view, x_sb, tt % 2, ident)
                # KT projection for this window: KT[dk, 512*win:+512]
                for co in range(NCK):
                    pj = ps_pj.tile([128, 512], F32, tag="pj")
                    for ck in range(NCK):
                        nc.tensor.matmul(
                            pj[:],
                            lhsT=wk_r[:, C * ck + 128 * co: C * ck + 128 * co + 128],
                            rhs=xkt[:, 512 * ck: 512 * ck + 512],
                            start=(ck == 0), stop=(ck == NCK - 1))
                    nc.scalar.activation(kt_sb[:, 2048 * co + 512 * win: 2048 * co + 512 * win + 512],
                                         pj[:], AFT.Identity, bias=bk_sb[:, co:co + 1])
                # V projection for the 4 tiles in this window (no bias)
                for tt in range(4):
                    t_glob = 4 * win + tt
                    for half in range(2):
                        n0, nn = (0, 512) if half == 0 else (512, 256)
                        pj = ps_pj.tile([128, 512], F32, tag="pj")
                        for ck in range(NCK):
                            nc.tensor.matmul(
                                pj[:, 0:nn],
                                lhsT=xkt[:, 512 * ck + 128 * tt: 512 * ck + 128 * tt + 128],
                                rhs=wv_r[:, C * ck + n0: C * ck + n0 + nn],
                                start=(ck == 0), stop=(ck == NCK - 1))
                        if half == 0:
                            nc.vector.tensor_copy(v_sb[:, 770 * t_glob: 770 * t_glob + 512], pj[:, 0:512])
                        else:
                            nc.scalar.activation(v_sb[:, 770 * t_glob + 512: 770 * t_glob + 768],
                                                 pj[:, 0:256], AFT.Identity)
                    nc.vector.tensor_copy(v_sb[:, 770 * t_glob + 768: 770 * t_glob + 770], onez[:])

        # ---------------- phase Q: quarters of queries ----------------
        with tc.tile_pool(name="xqst", bufs=2) as xqst, \
             tc.tile_pool(name="xqt", bufs=2) as xqtp, \
             tc.tile_pool(name="qt", bufs=1) as qtp, \
             tc.tile_pool(name="pt", bufs=3) as ptp, \
             tc.tile_pool(name="ob", bufs=2) as obp:
            for widx in range(8):       # 512-query windows
                qt_sb = qtp.tile([128, NCK * 512], F32R, tag="qt")
                xqt = xqtp.tile([128, NCK * 512], F32R, tag="xqtw")
                for tt in range(4):
                    r0 = 512 * widx + 128 * tt
                    x_sb = xqst.tile([128, C], F32, tag="xq")
                    nc.scalar.dma_start(x_sb[:], xq[r0:r0 + 128, :])
                    xt_view = xqt[:].rearrange("p (k n) -> p k n", k=NCK)[:, :, 128 * tt:128 * tt + 128]
                    _transpose_block(nc, ps_pj, xt_view, x_sb, tt % 2, ident)
                for co in range(NCK):
                    pj = ps_pj.tile([128, 512], F32, tag="pj")
                    for ck in range(NCK):
                        nc.tensor.matmul(
                            pj[:],
                            lhsT=wq_r[:, C * ck + 128 * co: C * ck + 128 * co + 128],
                            rhs=xqt[:, 512 * ck: 512 * ck + 512],
                            start=(ck == 0), stop=(ck == NCK - 1))
                    nc.scalar.activation(qt_sb[:, 512 * co: 512 * co + 512],
                                         pj[:], AFT.Identity, bias=bq_sb[:, co:co + 1])
                # flash over i-blocks of 256
                for al in range(2):
                    a = 2 * widx + al        # global i-block; queries 256a..256a+255
                    o_ps = []
                    for s2 in range(2):
                        oa_t = ps_o.tile([128, 512], F32, tag=f"oa{s2}", name=f"oa{s2}_{a}")
                        ob_t = ps_o.tile([128, 258], F32, tag=f"ob{s2}", name=f"ob{s2}_{a}")
                        o_ps.append((oa_t, ob_t))
                    for t in range(a + 1):   # local key tiles (global tile 2t+m)
                        st = ps_st.tile([128, 256], F32, tag="st")
                        for dk in range(NCK):
                            nc.tensor.matmul(
                                st[:],
                                lhsT=kt_sb[:, 2048 * dk + 128 * t: 2048 * dk + 128 * t + 128],
                                rhs=qt_sb[:, 512 * dk + 256 * al: 512 * dk + 256 * al + 256],
                                start=(dk == 0), stop=(dk == NCK - 1))
                        pt = ptp.tile([128, 256], F32R, tag="pt")
                        nc.scalar.activation(pt[:], st[:], AFT.Exp, scale=SCALE)
                        if t == a:
                            nc.vector.tensor_mul(pt[:], pt[:], mask_sb[:])
                        for s2 in range(2):
                            oa, ob = o_ps[s2]
                            nc.tensor.matmul(oa[:], lhsT=pt[:, 128 * s2:128 * s2 + 128],
                                             rhs=v_sb[:, 770 * t:770 * t + 512],
                                             start=(t == 0), stop=(t == a))
                            nc.tensor.matmul(ob[:], lhsT=pt[:, 128 * s2:128 * s2 + 128],
                                             rhs=v_sb[:, 770 * t + 512:770 * t + 770],
                                             start=(t == 0), stop=(t == a))
                    for s2 in range(2):
                        oa, ob = o_ps[s2]
                        o_sb = obp.tile([128, 770], F32, tag="osb")
                        nc.vector.tensor_copy(o_sb[:, 0:512], oa[:])
                        nc.scalar.activation(o_sb[:, 512:770], ob[:], AFT.Identity)
                        nc.sync.dma_start(out[256 * a + 128 * s2: 256 * a + 128 * s2 + 128, :], o_sb[:, 0:769])

    nc.compile()
    return nc


def _build_mask(m):
    jl = np.arange(128)[:, None]
    il = np.arange(256)[None, :]
    return (il >= jl + 128 * m).astype(np.float32)


def kernel(input, Wq, bq, Wk, bk, Wv, bv):
    global last_exec_time_ns, last_results
    x = np.ascontiguousarray(np.asarray(input, dtype=np.float32))
    Wq = np.ascontiguousarray(np.asarray(Wq, dtype=np.float32))
    Wk = np.ascontiguousarray(np.asarray(Wk, dtype=np.float32))
    Wv = np.ascontiguousarray(np.asarray(Wv, dtype=np.float32))
    bq = np.ascontiguousarray(np.asarray(bq, dtype=np.float32))
    bk = np.ascontiguousarray(np.asarray(bk, dtype=np.float32))
    bv_np = np.ascontiguousarray(np.asarray(bv, dtype=np.float32))

    if "nc" not in _nc_cache:
        _nc_cache["nc"] = build_module()
    nc = _nc_cache["nc"]

    masks = [_build_mask(m) for m in range(2)]
    key_rows = [np.concatenate([np.arange(128 * (2 * t + m), 128 * (2 * t + m) + 128)
                                for t in range(NKT)]) for m in range(2)]
    in_maps = []
    for core in range(8):
        b, m = core // 2, core % 2
        in_maps.append({
            "xq": x[b],
            "xk": np.ascontiguousarray(x[b][key_rows[m]]),
            "wq": Wq, "wk": Wk, "wv": Wv, "bq": bq, "bk": bk,
            "msk": masks[m],
        })

    trace = bool(int(__import__("os").environ.get("KERNEL_TRACE", "0")))
    res = run_bass_kernel_spmd(nc, in_maps, core_ids=list(range(8)), trace=trace)
    last_exec_time_ns = res.exec_time_ns
    last_results = res

    y = np.empty((B, T, C), dtype=np.float32)
    for b in range(B):
        o0 = res.results[2 * b]["out"]
        o1 = res.results[2 * b + 1]["out"]
        O = o0[:, :C].astype(np.float64) + o1[:, :C].astype(np.float64)
        l = o0[:, C].astype(np.float64) + o1[:, C].astype(np.float64)
        y[b] = (O / l[:, None] + bv_np.astype(np.float64)).astype(np.float32)
    return y


# revision 12
# speedup vs baseline: 1.0596x; 1.0442x over previous
"""Trainium2 Bass kernel for single-head causal attention.

Problem: B=4, T=4096, C=768, fp32.
  Q = x@Wq+bq; K = x@Wk+bk; V = x@Wv+bv
  out = softmax(causal(Q K^T / sqrt(C))) @ V

Sharding (8 cores): 2 cores per batch element. Each core processes ALL 4096
queries of its batch but only HALF the key tiles (128-row tiles, interleaved
by parity m = core%2). This makes the instruction stream identical across
cores (required for SPMD: one NEFF, data-only differences) and splits the
causal flash-attention work exactly 50/50 at i-block granularity of 256.

Each core returns unnormalized O_m = sum_j exp(s_ij) v_j and l_m = sum_j
exp(s_ij) (ones-column trick appended to V). Host combines:
  out = (O_0 + O_1) / (l_0 + l_1) + bv
(bv folds out of the attention average since softmax rows sum to 1;
no max-subtraction needed: |scores| <= ~5 so exp is well-conditioned.)

Matmuls run in float32r (TF32-class, ~1e-4 rel err, full PE rate at
free-dim >= 256). Producers must round to f32r explicitly.
"""
import sys

sys.path.insert(0, "/opt/trn_rl_repo")

import numpy as np
from contextlib import ExitStack

import concourse.bass as bass
import concourse.bacc as bacc
import concourse.mybir as mybir
import concourse.tile as tile
from concourse.bass_utils import run_bass_kernel_spmd
from concourse.masks import make_identity

dt = mybir.dt
F32, F32R = dt.float32, dt.float32r
AFT = mybir.ActivationFunctionType

B, T, C = 4, 4096, 768
NCK = C // 128            # 6 contraction tiles
NKT = T // 2 // 128       # 16 key tiles per core
NQ4 = T // 4              # 1024 queries per quarter-pass
SCALE = 1.0 / float(np.sqrt(np.float32(C)))

_nc_cache = {}
last_exec_time_ns = None
last_results = None


def _transpose_block(nc, ps_tr, xt_dst, x_src, eng_sel, ident):
    """PE-transpose x_src [128,768] f32 -> xt_dst view [128, 6, 128] f32r.

    xt_dst is an AP [128, 6, 128] (plane-strided dest). Routes the two
    PSUM evictions to alternating engines via eng_sel (0/1).
    """
    pt = ps_tr.tile([128, 512], F32, tag="pj", name="trp")
    for k in range(4):
        nc.tensor.matmul(pt[:, 128 * k:128 * k + 128],
                         lhsT=x_src[:, 128 * k:128 * k + 128], rhs=ident[:],
                         is_transpose=True, start=(k == 0), stop=(k == 3))
    if eng_sel == 0:
        nc.scalar.activation(xt_dst[:, 0:4, :], pt[:].rearrange("p (k f) -> p k f", k=4), AFT.Identity)
    else:
        nc.vector.tensor_copy(xt_dst[:, 0:4, :], pt[:].rearrange("p (k f) -> p k f", k=4))
    pt2 = ps_tr.tile([128, 512], F32, tag="pj", name="trp2")
    for k in range(2):
        nc.tensor.matmul(pt2[:, 128 * k:128 * k + 128],
                         lhsT=x_src[:, 512 + 128 * k:512 + 128 * k + 128], rhs=ident[:],
                         is_transpose=True, start=(k == 0), stop=(k == 1))
    if eng_sel == 0:
        nc.vector.tensor_copy(xt_dst[:, 4:6, :], pt2[:, 0:256].rearrange("p (k f) -> p k f", k=2))
    else:
        nc.scalar.activation(xt_dst[:, 4:6, :], pt2[:, 0:256].rearrange("p (k f) -> p k f", k=2), AFT.Identity)


def build_module():
    nc = bacc.Bacc("TRN2", target_bir_lowering=False, debug=False)

    xq = nc.dram_tensor("xq", [T, C], F32, kind="ExternalInput").ap()
    xk = nc.dram_tensor("xk", [T // 2, C], F32, kind="ExternalInput").ap()
    wq = nc.dram_tensor("wq", [C, C], F32, kind="ExternalInput").ap()
    wk = nc.dram_tensor("wk", [C, C], F32, kind="ExternalInput").ap()
    wv = nc.dram_tensor("wv", [C, C], F32, kind="ExternalInput").ap()
    bq = nc.dram_tensor("bq", [C], F32, kind="ExternalInput").ap()
    bk = nc.dram_tensor("bk", [C], F32, kind="ExternalInput").ap()
    msk = nc.dram_tensor("msk", [128, 256], F32, kind="ExternalInput").ap()
    out = nc.dram_tensor("out", [T, C + 1], F32, kind="ExternalOutput").ap()

    with tile.TileContext(nc) as tc, ExitStack() as ctx:
        const = ctx.enter_context(tc.tile_pool(name="const", bufs=1))
        ident = const.tile([128, 128], F32)
        make_identity(nc, ident[:])
        mask_sb = const.tile([128, 256], F32R)
        bq_sb = const.tile([128, NCK], F32)
        nc.sync.dma_start(bq_sb[:], bq.rearrange("(k p) -> p k", p=128))
        bk_sb = const.tile([128, NCK], F32)
        nc.sync.dma_start(bk_sb[:], bk.rearrange("(k p) -> p k", p=128))
        onez = const.tile([128, 2], F32)
        nc.vector.memset(onez[:, 0:1], 1.0)
        nc.vector.memset(onez[:, 1:2], 0.0)

        # --- weights: load fp32, round to f32r, plane layout [128, ck, 768]
        wq_pool = ctx.enter_context(tc.tile_pool(name="wq", bufs=1))
        wq_r = wq_pool.tile([128, NCK * C], F32R)
        wkv_pool = ctx.enter_context(tc.tile_pool(name="wkv", bufs=1))
        wk_r = wkv_pool.tile([128, NCK * C], F32R)
        wv_r = wkv_pool.tile([128, NCK * C], F32R)
        with tc.tile_pool(name="wstage", bufs=1) as wstage:
            mf = wstage.tile([128, 256], F32)
            nc.sync.dma_start(mf[:], msk[:])
            nc.vector.tensor_copy(mask_sb[:], mf[:])
            for w_dram, w_dst in ((wq, wq_r), (wk, wk_r), (wv, wv_r)):
                w_f32 = wstage.tile([128, NCK * C], F32, tag="wst")
                nc.sync.dma_start(
                    w_f32[:].rearrange("p (k n) -> p k n", k=NCK),
                    w_dram.rearrange("(k p) n -> p k n", p=128))
                nc.vector.tensor_copy(w_dst[:], w_f32[:])

        # --- PSUM pools (8 banks total):
        # tr: 1 bank, proj: 2, st: 1, O: 4
        kt_pool = ctx.enter_context(tc.tile_pool(name="kt", bufs=1))
        v_pool = ctx.enter_context(tc.tile_pool(name="v", bufs=1))
        # KT planes: [128, dk, 2048]; V tiles: [128, t, 769] (col 768 = ones)
        kt_sb = kt_pool.tile([128, NCK * 2048], F32R)
        v_sb = v_pool.tile([128, NKT * 770], F32R)

        # ---------------- phase K: keys -> KT, V ----------------
        with tc.tile_pool(name="xkst", bufs=2) as xkst, \
             tc.tile_pool(name="xkt", bufs=2) as xktp, \
             tc.tile_pool(name="ps_k", bufs=4, space="PSUM") as ps_k:
            for win in range(4):        # 512-key windows
                xkt = xktp.tile([128, NCK * 512], F32R, tag="xktw")
                for tt in range(4):     # 128-key tiles in window
                    x_sb = xkst.tile([128, C], F32, tag="xk")
                    nc.scalar.dma_start(x_sb[:], xk[512 * win + 128 * tt: 512 * win + 128 * tt + 128, :])
                    xt_view = xkt[:].rearrange("p (k n) -> p k n", k=NCK)[:, :, 128 * tt:128 * tt + 128]
                    _transpose_block(nc, ps_k, xt_view, x_sb, tt % 2, ident)
                # KT projection for this window: KT[dk, 512*win:+512]
                for co in range(NCK):
                    pj = ps_k.tile([128, 512], F32, tag="pj")
                    for ck in range(NCK):
                        nc.tensor.matmul(
                            pj[:],
                            lhsT=wk_r[:, C * ck + 128 * co: C * ck + 128 * co + 128],
                            rhs=xkt[:, 512 * ck: 512 * ck + 512],
                            start=(ck == 0), stop=(ck == NCK - 1))
                    nc.scalar.activation(kt_sb[:, 2048 * co + 512 * win: 2048 * co + 512 * win + 512],
                                         pj[:], AFT.Identity, bias=bk_sb[:, co:co + 1])
                # V projection for the 4 tiles in this window (no bias)
                for tt in range(4):
                    t_glob = 4 * win + tt
                    for half in range(2):
                        n0, nn = (0, 512) if half == 0 else (512, 256)
                        pj = ps_k.tile([128, 512], F32, tag="pj")
                        for ck in range(NCK):
                            nc.tensor.matmul(
                                pj[:, 0:nn],
                                lhsT=xkt[:, 512 * ck + 128 * tt: 512 * ck + 128 * tt + 128],
                                rhs=wv_r[:, C * ck + n0: C * ck + n0 + nn],
                                start=(ck == 0), stop=(ck == NCK - 1))
                        if half == 0:
                            nc.vector.tensor_copy(v_sb[:, 770 * t_glob: 770 * t_glob + 512], pj[:, 0:512])
                        else:
                            nc.scalar.activation(v_sb[:, 770 * t_glob + 512: 770 * t_glob + 768],
                                                 pj[:, 0:256], AFT.Identity)
                    nc.vector.tensor_copy(v_sb[:, 770 * t_glob + 768: 770 * t_glob + 770], onez[:])

        # ---------------- phase Q: 512-query windows ----------------
        ps_pj = ctx.enter_context(tc.tile_pool(name="ps_pj", bufs=2, space="PSUM"))
        ps_st = ctx.enter_context(tc.tile_pool(name="ps_st", bufs=2, space="PSUM"))
        ps_o = ctx.enter_context(tc.tile_pool(name="ps_o", bufs=1, space="PSUM"))
        with tc.tile_pool(name="xqst", bufs=2) as xqst, \
             tc.tile_pool(name="xqt", bufs=2) as xqtp, \
             tc.tile_pool(name="qt", bufs=1) as qtp, \
             tc.tile_pool(name="pt", bufs=3) as ptp, \
             tc.tile_pool(name="ob", bufs=2) as obp:
            for widx in range(8):       # 512-query windows
                qt_sb = qtp.tile([128, NCK * 512], F32R, tag="qt")
                xqt = xqtp.tile([128, NCK * 512], F32R, tag="xqtw")
                for tt in range(4):
                    r0 = 512 * widx + 128 * tt
                    x_sb = xqst.tile([128, C], F32, tag="xq")
                    nc.scalar.dma_start(x_sb[:], xq[r0:r0 + 128, :])
                    xt_view = xqt[:].rearrange("p (k n) -> p k n", k=NCK)[:, :, 128 * tt:128 * tt + 128]
                    _transpose_block(nc, ps_pj, xt_view, x_sb, tt % 2, ident)
                for co in range(NCK):
                    pj = ps_pj.tile([128, 512], F32, tag="pj")
                    for ck in range(NCK):
                        nc.tensor.matmul(
                            pj[:],
                            lhsT=wq_r[:, C * ck + 128 * co: C * ck + 128 * co + 128],
                            rhs=xqt[:, 512 * ck: 512 * ck + 512],
                            start=(ck == 0), stop=(ck == NCK - 1))
                    nc.scalar.activation(qt_sb[:, 512 * co: 512 * co + 512],
                                         pj[:], AFT.Identity, bias=bq_sb[:, co:co + 1])
                # flash: i-blocks of 256 (a = 2*widx+al), j-tiles of 128.
                # Software-pipelined emission: ST(k+1) is emitted before the
                # exp-dependent AV(k) so the PE never waits on ACT.
                seq = [(al, t) for al in range(2) for t in range(2 * widx + al + 1)]

                def emit_st(al, t):
                    st = ps_st.tile([128, 256], F32, tag="st", name=f"st{widx}_{al}_{t}")
                    for dk in range(NCK):
                        nc.tensor.matmul(
                            st[:],
                            lhsT=kt_sb[:, 2048 * dk + 128 * t: 2048 * dk + 128 * t + 128],
                            rhs=qt_sb[:, 512 * dk + 256 * al: 512 * dk + 256 * al + 256],
                            start=(dk == 0), stop=(dk == NCK - 1))
                    return st

                st_tiles = {seq[0]: emit_st(*seq[0])}
                o_cur = {}
                for k, (al, t) in enumerate(seq):
                    a = 2 * widx + al
                    if t == 0:
                        o_cur[al] = []
                        for s2 in range(2):
                            oa_t = ps_o.tile([128, 512], F32, tag=f"oa{s2}", name=f"oa{s2}_{a}")
                            ob_t = ps_o.tile([128, 258], F32, tag=f"ob{s2}", name=f"ob{s2}_{a}")
                            o_cur[al].append((oa_t, ob_t))
                    st = st_tiles.pop((al, t))
                    pt = ptp.tile([128, 256], F32R, tag="pt", name=f"pt{widx}_{al}_{t}")
                    nc.scalar.activation(pt[:], st[:], AFT.Exp, scale=SCALE)
                    if t == a:
                        nc.vector.tensor_mul(pt[:], pt[:], mask_sb[:])
                    if k + 1 < len(seq):
                        st_tiles[seq[k + 1]] = emit_st(*seq[k + 1])
                    for s2 in range(2):
                        oa, ob = o_cur[al][s2]
                        nc.tensor.matmul(oa[:], lhsT=pt[:, 128 * s2:128 * s2 + 128],
                                         rhs=v_sb[:, 770 * t:770 * t + 512],
                                         start=(t == 0), stop=(t == a))
                        nc.tensor.matmul(ob[:], lhsT=pt[:, 128 * s2:128 * s2 + 128],
                                         rhs=v_sb[:, 770 * t + 512:770 * t + 770],
                                         start=(t == 0), stop=(t == a))
                    if t == a:
                        for s2 in range(2):
                            oa, ob = o_cur[al][s2]
                            o_sb = obp.tile([128, 770], F32, tag="osb", name=f"osb{a}_{s2}")
                            nc.vector.tensor_copy(o_sb[:, 0:512], oa[:])
                            nc.scalar.activation(o_sb[:, 512:770], ob[:], AFT.Identity)
                            nc.sync.dma_start(out[256 * a + 128 * s2: 256 * a + 128 * s2 + 128, :],
                                              o_sb[:, 0:769])

    nc.compile()
    return nc


def _build_mask(m):
    jl = np.arange(128)[:, None]
    il = np.arange(256)[None, :]
    return (il >= jl + 128 * m).astype(np.float32)


def kernel(input, Wq, bq, Wk, bk, Wv, bv):
    global last_exec_time_ns, last_results
    x = np.ascontiguousarray(np.asarray(input, dtype=np.float32))
    Wq = np.ascontiguousarray(np.asarray(Wq, dtype=np.float32))
    Wk = np.ascontiguousarray(np.asarray(Wk, dtype=np.float32))
    Wv = np.ascontiguousarray(np.asarray(Wv, dtype=np.float32))
    bq = np.ascontiguousarray(np.asarray(bq, dtype=np.float32))
    bk = np.ascontiguousarray(np.asarray(bk, dtype=np.float32))
    bv_np = np.ascontiguousarray(np.asarray(bv, dtype=np.float32))

    if "nc" not in _nc_cache:
        _nc_cache["nc"] = build_module()
    nc = _nc_cache["nc"]

    masks = [_build_mask(m) for m in range(2)]
    key_rows = [np.concatenate([np.arange(128 * (2 * t + m), 128 * (2 * t + m) + 128)
                                for t in range(NKT)]) for m in range(2)]
    in_maps = []
    for core in range(8):
        b, m = core // 2, core % 2
        in_maps.append({
            "xq": x[b],
            "xk": np.ascontiguousarray(x[b][key_rows[m]]),
            "wq": Wq, "wk": Wk, "wv": Wv, "bq": bq, "bk": bk,
            "msk": masks[m],
        })

    trace = bool(int(__import__("os").environ.get("KERNEL_TRACE", "0")))
    res = run_bass_kernel_spmd(nc, in_maps, core_ids=list(range(8)), trace=trace)
    last_exec_time_ns = res.exec_time_ns
    last_results = res

    y = np.empty((B, T, C), dtype=np.float32)
    for b in range(B):
        o0 = res.results[2 * b]["out"]
        o1 = res.results[2 * b + 1]["out"]
        O = o0[:, :C].astype(np.float64) + o1[:, :C].astype(np.float64)
        l = o0[:, C].astype(np.float64) + o1[:, C].astype(np.float64)
        y[b] = (O / l[:, None] + bv_np.astype(np.float64)).astype(np.float32)
    return y


# revision 13
# speedup vs baseline: 1.1814x; 1.1149x over previous
"""Trainium2 Bass kernel for single-head causal attention.

Problem: B=4, T=4096, C=768, fp32.
  Q = x@Wq+bq; K = x@Wk+bk; V = x@Wv+bv
  out = softmax(causal(Q K^T / sqrt(C))) @ V

Sharding (8 cores): 2 cores per batch element. Each core processes ALL 4096
queries of its batch but only HALF the key tiles (128-row tiles, interleaved
by parity m = core%2). This makes the instruction stream identical across
cores (required for SPMD: one NEFF, data-only differences) and splits the
causal flash-attention work exactly 50/50 at i-block granularity of 256.

Each core returns unnormalized O_m = sum_j exp(s_ij) v_j and l_m = sum_j
exp(s_ij) (ones-column trick appended to V). Host combines:
  out = (O_0 + O_1) / (l_0 + l_1) + bv
(bv folds out of the attention average since softmax rows sum to 1;
no max-subtraction needed: |scores| <= ~5 so exp is well-conditioned.)

Matmuls run in float32r (TF32-class, ~1e-4 rel err, full PE rate at
free-dim >= 256). Producers must round to f32r explicitly.
"""
import sys

sys.path.insert(0, "/opt/trn_rl_repo")

import numpy as np
from contextlib import ExitStack

import concourse.bass as bass
import concourse.bacc as bacc
import concourse.mybir as mybir
import concourse.tile as tile
from concourse.bass_utils import run_bass_kernel_spmd
from concourse.masks import make_identity

dt = mybir.dt
F32, F32R = dt.float32, dt.float32r
AFT = mybir.ActivationFunctionType

B, T, C = 4, 4096, 768
NCK = C // 128            # 6 contraction tiles
NKT = T // 2 // 128       # 16 key tiles per core
NQ4 = T // 4              # 1024 queries per quarter-pass
SCALE = 1.0 / float(np.sqrt(np.float32(C)))

_nc_cache = {}
last_exec_time_ns = None
last_results = None


def _transpose_block(nc, ps_tr, xt_dst, x_src, eng_sel, ident):
    """PE-transpose x_src [128,768] f32 -> xt_dst view [128, 6, 128] f32r.

    xt_dst is an AP [128, 6, 128] (plane-strided dest). Routes the two
    PSUM evictions to alternating engines via eng_sel (0/1).
    """
    pt = ps_tr.tile([128, 512], F32, tag="pj", name="trp")
    for k in range(4):
        nc.tensor.matmul(pt[:, 128 * k:128 * k + 128],
                         lhsT=x_src[:, 128 * k:128 * k + 128], rhs=ident[:],
                         is_transpose=True, start=(k == 0), stop=(k == 3))
    if eng_sel == 0:
        nc.scalar.activation(xt_dst[:, 0:4, :], pt[:].rearrange("p (k f) -> p k f", k=4), AFT.Identity)
    else:
        nc.vector.tensor_copy(xt_dst[:, 0:4, :], pt[:].rearrange("p (k f) -> p k f", k=4))
    pt2 = ps_tr.tile([128, 512], F32, tag="pj", name="trp2")
    for k in range(2):
        nc.tensor.matmul(pt2[:, 128 * k:128 * k + 128],
                         lhsT=x_src[:, 512 + 128 * k:512 + 128 * k + 128], rhs=ident[:],
                         is_transpose=True, start=(k == 0), stop=(k == 1))
    if eng_sel == 0:
        nc.vector.tensor_copy(xt_dst[:, 4:6, :], pt2[:, 0:256].rearrange("p (k f) -> p k f", k=2))
    else:
        nc.scalar.activation(xt_dst[:, 4:6, :], pt2[:, 0:256].rearrange("p (k f) -> p k f", k=2), AFT.Identity)


def build_module():
    nc = bacc.Bacc("TRN2", target_bir_lowering=False, debug=False)

    xq = nc.dram_tensor("xq", [T, C], F32, kind="ExternalInput").ap()
    xk = nc.dram_tensor("xk", [T // 2, C], F32, kind="ExternalInput").ap()
    wq = nc.dram_tensor("wq", [C, C], F32, kind="ExternalInput").ap()
    wk = nc.dram_tensor("wk", [C, C], F32, kind="ExternalInput").ap()
    wv = nc.dram_tensor("wv", [C, C], F32, kind="ExternalInput").ap()
    bq = nc.dram_tensor("bq", [C], F32, kind="ExternalInput").ap()
    bk = nc.dram_tensor("bk", [C], F32, kind="ExternalInput").ap()
    msk = nc.dram_tensor("msk", [128, 256], F32, kind="ExternalInput").ap()
    out = nc.dram_tensor("out", [T, C + 1], F32, kind="ExternalOutput").ap()

    with tile.TileContext(nc) as tc, ExitStack() as ctx:
        const = ctx.enter_context(tc.tile_pool(name="const", bufs=1))
        ident = const.tile([128, 128], F32)
        make_identity(nc, ident[:])
        mask_sb = const.tile([128, 256], F32R)
        bq_sb = const.tile([128, NCK], F32)
        nc.sync.dma_start(bq_sb[:], bq.rearrange("(k p) -> p k", p=128))
        bk_sb = const.tile([128, NCK], F32)
        nc.sync.dma_start(bk_sb[:], bk.rearrange("(k p) -> p k", p=128))
        onez = const.tile([128, 2], F32)
        nc.vector.memset(onez[:, 0:1], 1.0)
        nc.vector.memset(onez[:, 1:2], 0.0)

        # --- weights: load fp32, round to f32r, plane layout [128, ck, 768]
        wq_pool = ctx.enter_context(tc.tile_pool(name="wq", bufs=1))
        wq_r = wq_pool.tile([128, NCK * C], F32R)
        wkv_pool = ctx.enter_context(tc.tile_pool(name="wkv", bufs=1))
        wk_r = wkv_pool.tile([128, NCK * C], F32R)
        wv_r = wkv_pool.tile([128, NCK * C], F32R)
        with tc.tile_pool(name="wstage", bufs=1) as wstage:
            mf = wstage.tile([128, 256], F32)
            nc.sync.dma_start(mf[:], msk[:])
            nc.vector.tensor_copy(mask_sb[:], mf[:])
            for w_dram, w_dst in ((wk, wk_r), (wv, wv_r), (wq, wq_r)):
                w_f32 = wstage.tile([128, NCK * C], F32, tag="wst")
                nc.sync.dma_start(
                    w_f32[:].rearrange("p (k n) -> p k n", k=NCK),
                    w_dram.rearrange("(k p) n -> p k n", p=128))
                nc.vector.tensor_copy(w_dst[:], w_f32[:])

        # --- PSUM pools (8 banks total):
        # tr: 1 bank, proj: 2, st: 1, O: 4
        kt_pool = ctx.enter_context(tc.tile_pool(name="kt", bufs=1))
        v_pool = ctx.enter_context(tc.tile_pool(name="v", bufs=1))
        # KT planes: [128, dk, 2048]; V tiles: [128, t, 769] (col 768 = ones)
        kt_sb = kt_pool.tile([128, NCK * 2048], F32R)
        v_sb = v_pool.tile([128, NKT * 770], F32R)

        # ---------------- phase K: keys -> KT, V ----------------
        with tc.tile_pool(name="xkst", bufs=3) as xkst, \
             tc.tile_pool(name="xkt", bufs=2) as xktp, \
             tc.tile_pool(name="ps_k", bufs=4, space="PSUM") as ps_k:
            xk_tiles = {}

            def load_xk(win, tt):
                x_sb = xkst.tile([128, C], F32, tag="xk", name=f"xk{win}_{tt}")
                nc.scalar.dma_start(x_sb[:], xk[512 * win + 128 * tt: 512 * win + 128 * tt + 128, :])
                xk_tiles[(win, tt)] = x_sb

            for tt in range(4):
                load_xk(0, tt)
            for win in range(4):        # 512-key windows
                xkt = xktp.tile([128, NCK * 512], F32R, tag="xktw")
                for tt in range(4):     # 128-key tiles in window
                    x_sb = xk_tiles.pop((win, tt))
                    xt_view = xkt[:].rearrange("p (k n) -> p k n", k=NCK)[:, :, 128 * tt:128 * tt + 128]
                    _transpose_block(nc, ps_k, xt_view, x_sb, tt % 2, ident)
                    if win + 1 < 4:
                        load_xk(win + 1, tt)
                # KT projection for this window: KT[dk, 512*win:+512]
                for co in range(NCK):
                    pj = ps_k.tile([128, 512], F32, tag="pj")
                    for ck in range(NCK):
                        nc.tensor.matmul(
                            pj[:],
                            lhsT=wk_r[:, C * ck + 128 * co: C * ck + 128 * co + 128],
                            rhs=xkt[:, 512 * ck: 512 * ck + 512],
                            start=(ck == 0), stop=(ck == NCK - 1))
                    nc.scalar.activation(kt_sb[:, 2048 * co + 512 * win: 2048 * co + 512 * win + 512],
                                         pj[:], AFT.Identity, bias=bk_sb[:, co:co + 1])
                # V projection for the 4 tiles in this window (no bias)
                for tt in range(4):
                    t_glob = 4 * win + tt
                    for half in range(2):
                        n0, nn = (0, 512) if half == 0 else (512, 256)
                        pj = ps_k.tile([128, 512], F32, tag="pj")
                        for ck in range(NCK):
                            nc.tensor.matmul(
                                pj[:, 0:nn],
                                lhsT=xkt[:, 512 * ck + 128 * tt: 512 * ck + 128 * tt + 128],
                                rhs=wv_r[:, C * ck + n0: C * ck + n0 + nn],
                                start=(ck == 0), stop=(ck == NCK - 1))
                        if half == 0:
                            nc.vector.tensor_copy(v_sb[:, 770 * t_glob: 770 * t_glob + 512], pj[:, 0:512])
                        else:
                            nc.scalar.activation(v_sb[:, 770 * t_glob + 512: 770 * t_glob + 768],
                                                 pj[:, 0:256], AFT.Identity)
                    nc.vector.tensor_copy(v_sb[:, 770 * t_glob + 768: 770 * t_glob + 770], onez[:])

        # ---------------- phase Q: 512-query windows ----------------
        ps_pj = ctx.enter_context(tc.tile_pool(name="ps_pj", bufs=2, space="PSUM"))
        ps_st = ctx.enter_context(tc.tile_pool(name="ps_st", bufs=2, space="PSUM"))
        ps_o = ctx.enter_context(tc.tile_pool(name="ps_o", bufs=1, space="PSUM"))
        with tc.tile_pool(name="xqst", bufs=3) as xqst, \
             tc.tile_pool(name="xqt", bufs=2) as xqtp, \
             tc.tile_pool(name="qt", bufs=1) as qtp, \
             tc.tile_pool(name="pt", bufs=3) as ptp, \
             tc.tile_pool(name="ob", bufs=2) as obp:
            xq_tiles = {}

            def load_xq(widx, tt):
                x_sb = xqst.tile([128, C], F32, tag="xq", name=f"xq{widx}_{tt}")
                nc.sync.dma_start(x_sb[:], xq[512 * widx + 128 * tt: 512 * widx + 128 * tt + 128, :])
                xq_tiles[(widx, tt)] = x_sb

            for tt in range(4):
                load_xq(0, tt)
            for widx in range(8):       # 512-query windows
                qt_sb = qtp.tile([128, NCK * 512], F32R, tag="qt")
                xqt = xqtp.tile([128, NCK * 512], F32R, tag="xqtw")
                for tt in range(4):
                    x_sb = xq_tiles.pop((widx, tt))
                    xt_view = xqt[:].rearrange("p (k n) -> p k n", k=NCK)[:, :, 128 * tt:128 * tt + 128]
                    _transpose_block(nc, ps_pj, xt_view, x_sb, tt % 2, ident)
                    if widx + 1 < 8:
                        load_xq(widx + 1, tt)
                for co in range(NCK):
                    pj = ps_pj.tile([128, 512], F32, tag="pj")
                    for ck in range(NCK):
                        nc.tensor.matmul(
                            pj[:],
                            lhsT=wq_r[:, C * ck + 128 * co: C * ck + 128 * co + 128],
                            rhs=xqt[:, 512 * ck: 512 * ck + 512],
                            start=(ck == 0), stop=(ck == NCK - 1))
                    nc.scalar.activation(qt_sb[:, 512 * co: 512 * co + 512],
                                         pj[:], AFT.Identity, bias=bq_sb[:, co:co + 1])
                # flash: i-blocks of 256 (a = 2*widx+al), j-tiles of 128.
                # Software-pipelined emission: ST(k+1) is emitted before the
                # exp-dependent AV(k) so the PE never waits on ACT.
                seq = [(al, t) for al in range(2) for t in range(2 * widx + al + 1)]

                def emit_st(al, t):
                    st = ps_st.tile([128, 256], F32, tag="st", name=f"st{widx}_{al}_{t}")
                    for dk in range(NCK):
                        nc.tensor.matmul(
                            st[:],
                            lhsT=kt_sb[:, 2048 * dk + 128 * t: 2048 * dk + 128 * t + 128],
                            rhs=qt_sb[:, 512 * dk + 256 * al: 512 * dk + 256 * al + 256],
                            start=(dk == 0), stop=(dk == NCK - 1))
                    return st

                st_tiles = {seq[0]: emit_st(*seq[0])}
                o_cur = {}
                for k, (al, t) in enumerate(seq):
                    a = 2 * widx + al
                    if t == 0:
                        o_cur[al] = []
                        for s2 in range(2):
                            oa_t = ps_o.tile([128, 512], F32, tag=f"oa{s2}", name=f"oa{s2}_{a}")
                            ob_t = ps_o.tile([128, 258], F32, tag=f"ob{s2}", name=f"ob{s2}_{a}")
                            o_cur[al].append((oa_t, ob_t))
                    st = st_tiles.pop((al, t))
                    pt = ptp.tile([128, 256], F32R, tag="pt", name=f"pt{widx}_{al}_{t}")
                    nc.scalar.activation(pt[:], st[:], AFT.Exp, scale=SCALE)
                    if t == a:
                        nc.vector.tensor_mul(pt[:], pt[:], mask_sb[:])
                    if k + 1 < len(seq):
                        st_tiles[seq[k + 1]] = emit_st(*seq[k + 1])
                    for s2 in range(2):
                        oa, ob = o_cur[al][s2]
                        nc.tensor.matmul(oa[:], lhsT=pt[:, 128 * s2:128 * s2 + 128],
                                         rhs=v_sb[:, 770 * t:770 * t + 512],
                                         start=(t == 0), stop=(t == a))
                        nc.tensor.matmul(ob[:], lhsT=pt[:, 128 * s2:128 * s2 + 128],
                                         rhs=v_sb[:, 770 * t + 512:770 * t + 770],
                                         start=(t == 0), stop=(t == a))
                    if t == a:
                        for s2 in range(2):
                            oa, ob = o_cur[al][s2]
                            o_sb = obp.tile([128, 770], F32, tag="osb", name=f"osb{a}_{s2}")
                            nc.vector.tensor_copy(o_sb[:, 0:512], oa[:])
                            nc.scalar.activation(o_sb[:, 512:770], ob[:], AFT.Identity)
                            nc.sync.dma_start(out[256 * a + 128 * s2: 256 * a + 128 * s2 + 128, :],
                                              o_sb[:, 0:769])

    nc.compile()
    return nc


def _build_mask(m):
    jl = np.arange(128)[:, None]
    il = np.arange(256)[None, :]
    return (il >= jl + 128 * m).astype(np.float32)


def kernel(input, Wq, bq, Wk, bk, Wv, bv):
    global last_exec_time_ns, last_results
    x = np.ascontiguousarray(np.asarray(input, dtype=np.float32))
    Wq = np.ascontiguousarray(np.asarray(Wq, dtype=np.float32))
    Wk = np.ascontiguousarray(np.asarray(Wk, dtype=np.float32))
    Wv = np.ascontiguousarray(np.asarray(Wv, dtype=np.float32))
    bq = np.ascontiguousarray(np.asarray(bq, dtype=np.float32))
    bk = np.ascontiguousarray(np.asarray(bk, dtype=np.float32))
    bv_np = np.ascontiguousarray(np.asarray(bv, dtype=np.float32))

    if "nc" not in _nc_cache:
        _nc_cache["nc"] = build_module()
    nc = _nc_cache["nc"]

    masks = [_build_mask(m) for m in range(2)]
    key_rows = [np.concatenate([np.arange(128 * (2 * t + m), 128 * (2 * t + m) + 128)
                                for t in range(NKT)]) for m in range(2)]
    in_maps = []
    for core in range(8):
        b, m = core // 2, core % 2
        in_maps.append({
            "xq": x[b],
            "xk": np.ascontiguousarray(x[b][key_rows[m]]),
            "wq": Wq, "wk": Wk, "wv": Wv, "bq": bq, "bk": bk,
            "msk": masks[m],
        })

    trace = bool(int(__import__("os").environ.get("KERNEL_TRACE", "0")))
    res = run_bass_kernel_spmd(nc, in_maps, core_ids=list(range(8)), trace=trace)
    last_exec_time_ns = res.exec_time_ns
    last_results = res

    y = np.empty((B, T, C), dtype=np.float32)
    for b in range(B):
        o0 = res.results[2 * b]["out"]
        o1 = res.results[2 * b + 1]["out"]
        O = o0[:, :C].astype(np.float64) + o1[:, :C].astype(np.float64)
        l = o0[:, C].astype(np.float64) + o1[:, C].astype(np.float64)
        y[b] = (O / l[:, None] + bv_np.astype(np.float64)).astype(np.float32)
    return y
